# revision 11
# baseline (speedup 1.0000x reference)
"""GaussianVectorQuantizer forward (training path) on 8 Trainium2 NeuronCores.

Strategy (data-parallel over batch B=16, 2 batches per core, no collectives):

  reference math (TEMP=0.5, pq = precision_q = 0.5/(1+e^{log_param_q})):
    c_probs = softmax((c_logits*pq_cls + g1)/TEMP)            (host: 16x16, needs jax PRNG)
    logit_all[b,n,k,s] = -(|ze|^2 - 2 ze.books + |book|^2)*pq
    logits[b,n,s] = sum_k c_probs[b,k] * logit_all[b,n,k,s]
    enc = softmax_s((logit_all + g2)/TEMP)
    zq[b,n,d] = sum_{k,s} enc * books * c_probs
    prob/log_prob = softmax/log_softmax over s of logits

  Key algebra used by the device kernel:
   - softmax over s is invariant to per-(b,n[,k]) shifts => the |ze|^2 term
     cancels everywhere (logits are only consumed through softmax/log_softmax).
   - enc exponent: (logit_all + g2)/TEMP = 4pq*cross + (2*g2 - 2pq*b2) + const(n)
     The parenthesised "biasc" term is precomputed on host (g2 needs the jax
     threefry PRNG; it is input-independent noise with a fixed key).
   - logits = 2pq*(ze @ wbooks) - pq*wb2 + const(n), where wbooks/wb2 are the
     c_probs-weighted codebook contractions (tiny host matmuls).
   - zq = sum_k (c_bk / Z_nk) * (e_k @ books_k) with e = exp(4pq*cross + biasc - max).

  Device pipeline per (batch b, cluster k):
    PE:  cross = (4pq*zeT).T @ booksT          (f32r matmuls, (128n x 512s) tiles)
    DVE: E = cross + biasc ; rowmax (fused tensor_tensor_reduce)
    ACT: e = exp(E - max), accumulating Z = sum_s e
    DVE: es = e * (c_bk/Z)  -> bf16
    PE:  transpose es -> esT (bf16)
    ACT: copy esT PSUM->SBUF
    PE:  zqT[d,n] += books_k[s,d].T @ esT[s,n]  (bf16, accumulated over k in PSUM)
  plus a small per-b logits phase (matmul + softmax/log-softmax).
"""

import sys

if "/opt/trn_rl_repo" not in sys.path:
    sys.path.insert(0, "/opt/trn_rl_repo")

import numpy as np

B, N, D, K, S = 16, 1024, 256, 16, 512
TEMP = 0.5
EPS = 1e-10
NCORES = 8
BL = B // NCORES  # batches per core
NCH = N // 128    # n-chunks of 128

_CACHE = {}


# ---------------------------------------------------------------- device code
def build_bass(BL=BL, NCH=NCH, K=K, S=S, D=D, use_f32r=True, zq_bf16=True):
    """Build + compile the per-core Bass program (SPMD; identical on all cores)."""
    from contextlib import ExitStack

    import concourse.bass as bass
    import concourse.tile as tile
    from concourse import bacc, mybir
    from concourse.masks import make_identity

    f32 = mybir.dt.float32
    f32r = mybir.dt.float32r
    bf16 = mybir.dt.bfloat16
    mdt = bf16 if zq_bf16 else f32
    mmf = f32r if use_f32r else f32
    AX = mybir.AxisListType.X
    OP = mybir.AluOpType
    AF = mybir.ActivationFunctionType

    Nloc = NCH * 128
    SC = S // 128   # s-chunks per cluster
    NW = min(512, Nloc)  # zq matmul moving-operand width
    NH = Nloc // NW
    ZW = max(Nloc, 512)  # per-dh stride in the zq PSUM accumulator (bank-aligned)
    DH = D // 128   # d-halves

    nc = bacc.Bacc("TRN2", target_bir_lowering=False, debug=False)

    zeT_h = nc.dram_tensor("zeT", [BL, DH, 128, Nloc], mmf, kind="ExternalInput")
    booksT_h = nc.dram_tensor("booksT", [DH, 128, K * S], mmf, kind="ExternalInput")
    booksN_h = nc.dram_tensor("booksN", [K, SC, 128, D], mdt, kind="ExternalInput")
    biasc_h = nc.dram_tensor("biasc", [BL, K, 128, NCH, S], f32, kind="ExternalInput")
    wbw_h = nc.dram_tensor("wbw", [BL, DH, 128, S], f32, kind="ExternalInput")
    wbrow_h = nc.dram_tensor("wbrow", [BL, S], f32, kind="ExternalInput")
    cvec_h = nc.dram_tensor("cvec", [BL, 128, K], f32, kind="ExternalInput")
    ones_h = nc.dram_tensor("ones", [1, 128], f32, kind="ExternalInput")
    zq_h = nc.dram_tensor("zq", [BL, Nloc, D], f32, kind="ExternalOutput")
    prob_h = nc.dram_tensor("prob", [BL, Nloc, S], f32, kind="ExternalOutput")
    logp_h = nc.dram_tensor("log_prob", [BL, Nloc, S], f32, kind="ExternalOutput")

    def mmdt(ap):
        return ap

    with tile.TileContext(nc) as tc:
        with ExitStack() as ctx:
            consts = ctx.enter_context(tc.tile_pool(name="consts", bufs=1))
            biasc_p = ctx.enter_context(tc.tile_pool(name="biasc", bufs=2))
            esT_p = ctx.enter_context(tc.tile_pool(name="esT", bufs=2))
            work_p = ctx.enter_context(tc.tile_pool(name="work", bufs=3))
            small_p = ctx.enter_context(tc.tile_pool(name="small", bufs=12))
            out_p = ctx.enter_context(tc.tile_pool(name="outs", bufs=2))
            mm_ps = ctx.enter_context(tc.tile_pool(name="mm_ps", bufs=2, space="PSUM"))
            eT_ps = ctx.enter_context(tc.tile_pool(name="eT_ps", bufs=2, space="PSUM"))
            zq_ps = ctx.enter_context(tc.tile_pool(name="zq_ps", bufs=1, space="PSUM"))

            # ---- constants into SBUF
            booksT_sb = consts.tile([128, DH, K * S], mmf)
            nc.sync.dma_start(booksT_sb, booksT_h.ap().rearrange("dc p ks -> p dc ks"))
            booksN_sb = consts.tile([128, K, SC, D], mdt)
            nc.sync.dma_start(booksN_sb, booksN_h.ap().rearrange("k sc p d -> p k sc d"))
            zeT_sb = consts.tile([128, BL, DH, Nloc], mmf)
            nc.sync.dma_start(zeT_sb, zeT_h.ap().rearrange("b dc p n -> p b dc n"))
            wbw_sb = consts.tile([128, BL, DH, S], f32)
            nc.sync.dma_start(wbw_sb, wbw_h.ap().rearrange("b dc p s -> p b dc s"))
            wbrow_sb = consts.tile([1, BL, S], f32)
            nc.sync.dma_start(wbrow_sb, wbrow_h.ap().rearrange("b s -> () b s"))
            cvec_sb = consts.tile([128, BL, K], f32)
            nc.sync.dma_start(cvec_sb, cvec_h.ap().rearrange("b p k -> p b k"))
            ones_sb = consts.tile([1, 128], f32)
            nc.sync.dma_start(ones_sb, ones_h.ap())
            identf = consts.tile([128, 128], f32)
            make_identity(nc, identf)
            identm = consts.tile([128, 128], mdt)
            nc.vector.tensor_copy(identm, identf)

            for b in range(BL):
                # ---------------- logits / prob / log_prob phase
                for nch in range(NCH):
                    arg = mm_ps.tile([128, S], f32, tag="mm")
                    for dc in range(DH):
                        nc.tensor.matmul(
                            arg,
                            zeT_sb[:, b, dc, nch * 128:(nch + 1) * 128].bitcast(f32),
                            wbw_sb[:, b, dc, :],
                            start=(dc == 0),
                            stop=False,
                        )
                    nc.tensor.matmul(
                        arg,
                        ones_sb[:1, :],
                        wbrow_sb[:1, b, :],
                        start=False,
                        stop=True,
                    )
                    nmxp = small_p.tile([128, 1], f32, tag="sm")
                    nc.vector.tensor_reduce(
                        nmxp, arg, axis=AX, op=OP.max, negate=True
                    )
                    ep = out_p.tile([128, S], f32, tag="ep")
                    Zp = small_p.tile([128, 1], f32, tag="sm")
                    nc.scalar.activation(ep, arg, AF.Exp, bias=nmxp, accum_out=Zp)
                    rZp = small_p.tile([128, 1], f32, tag="sm")
                    nc.vector.reciprocal(rZp, Zp)
                    nc.vector.tensor_scalar_mul(ep, ep, rZp)
                    nc.sync.dma_start(prob_h[b, nch * 128:(nch + 1) * 128, :], ep)
                    lnZ = small_p.tile([128, 1], f32, tag="sm")
                    nc.scalar.activation(lnZ, Zp, AF.Ln)
                    off = small_p.tile([128, 1], f32, tag="sm")
                    nc.vector.tensor_sub(off, lnZ, nmxp)
                    logpt = out_p.tile([128, S], f32, tag="logpt")
                    nc.vector.tensor_scalar(logpt, arg, off, None, op0=OP.subtract)
                    nc.sync.dma_start(logp_h[b, nch * 128:(nch + 1) * 128, :], logpt)

                # ---------------- enc + zq phase
                zqT = zq_ps.tile([128, DH * ZW], f32, tag="zqT")
                for k in range(K):
                    nhalf = max(NCH // 2, 1)
                    bias_tiles = []
                    for h in range((NCH + nhalf - 1) // nhalf):
                        bia = biasc_p.tile([128, nhalf, S], f32, tag="bia")
                        nc.sync.dma_start(
                            bia, biasc_h[b, k][:, h * nhalf:(h + 1) * nhalf, :]
                        )
                        bias_tiles.append(bia)
                    esT = esT_p.tile([128, SC, Nloc], mdt, tag="esT")
                    for nch in range(NCH):
                        cr = mm_ps.tile([128, S], f32, tag="mm")
                        for dc in range(DH):
                            nc.tensor.matmul(
                                cr,
                                mmdt(zeT_sb[:, b, dc, nch * 128:(nch + 1) * 128]),
                                mmdt(booksT_sb[:, dc, k * S:(k + 1) * S]),
                                start=(dc == 0),
                                stop=(dc == DH - 1),
                            )
                        E = work_p.tile([128, S], f32, tag="E")
                        nc.vector.tensor_add(
                            E, cr, bias_tiles[nch // nhalf][:, nch % nhalf, :]
                        )
                        nmx = small_p.tile([128, 1], f32, tag="sm")
                        nc.vector.tensor_reduce(
                            nmx, E, axis=AX, op=OP.max, negate=True
                        )
                        e = work_p.tile([128, S], f32, tag="e")
                        Z = small_p.tile([128, 1], f32, tag="sm")
                        nc.scalar.activation(e, E, AF.Exp, bias=nmx, accum_out=Z)
                        rZ = small_p.tile([128, 1], f32, tag="sm")
                        nc.vector.reciprocal(rZ, Z)
                        fsc = small_p.tile([128, 1], f32, tag="sm")
                        nc.vector.tensor_mul(fsc, rZ, cvec_sb[:, b, k:k + 1])
                        es = work_p.tile([128, S], mdt, tag="es")
                        nc.vector.tensor_scalar_mul(es, e, fsc)
                        eT = eT_ps.tile([128, S], mdt, tag="eT")
                        for c in range(SC):
                            nc.tensor.transpose(
                                eT[:, c * 128:(c + 1) * 128],
                                es[:, c * 128:(c + 1) * 128],
                                identm,
                            )
                        nc.scalar.activation(
                            esT[:, :, nch * 128:(nch + 1) * 128],
                            eT.rearrange("p (c n) -> p c n", c=SC),
                            AF.Copy,
                        )
                    for dh in range(DH):
                        for sc in range(SC):
                            lhsT = booksN_sb[:, k, sc, dh * 128:(dh + 1) * 128]
                            for nh in range(NH):
                                nc.tensor.matmul(
                                    zqT[:, dh * ZW + nh * NW: dh * ZW + (nh + 1) * NW],
                                    lhsT,
                                    esT[:, sc, nh * NW:(nh + 1) * NW],
                                    start=(k == 0 and sc == 0),
                                    stop=(k == K - 1 and sc == SC - 1),
                                )

                # ---------------- zq transpose + store
                for nch in range(NCH):
                    zqb = work_p.tile([128, DH * 128], f32, tag="zqb")
                    for dh in range(DH):
                        nc.vector.tensor_copy(
                            zqb[:, dh * 128:(dh + 1) * 128],
                            zqT[:, dh * ZW + nch * 128: dh * ZW + (nch + 1) * 128],
                        )
                    zqf = mm_ps.tile([128, D], f32, tag="mm")
                    for dh in range(DH):
                        nc.tensor.transpose(
                            zqf[:, dh * 128:(dh + 1) * 128],
                            zqb[:, dh * 128:(dh + 1) * 128],
                            identf,
                        )
                    zqo = out_p.tile([128, D], f32, tag="zqo")
                    nc.vector.tensor_copy(zqo, zqf)
                    nc.sync.dma_start(zq_h[b, nch * 128:(nch + 1) * 128, :], zqo)

    nc.compile()
    return nc


# ---------------------------------------------------------------- host prep
def host_prep(ze, c_logits, books, log_param_q, log_param_q_cls):
    """Reproduce the reference's PRNG/noise + precompute device input layouts."""
    import jax
    import jax.numpy as jnp
    import ml_dtypes

    cpu = jax.local_devices(backend="cpu")[0]

    def _gumbel(key, shape):
        u = jax.random.uniform(key, shape, dtype=jnp.float32)
        return -jnp.log(-jnp.log(u + EPS) + EPS)

    with jax.default_device(cpu):
        lpq = jnp.asarray(log_param_q, jnp.float32)
        lpqc = jnp.asarray(log_param_q_cls, jnp.float32)
        param_q = 1.0 + jnp.exp(lpq)
        precision_q = 0.5 / jnp.clip(param_q, 1e-10)
        param_q_cls = 1.0 + jnp.exp(lpqc)
        precision_q_cls = 0.5 / jnp.clip(param_q_cls, 1e-10)

        gkey = jax.random.key(42)
        kg1, kg2 = jax.random.split(gkey)
        c_probs = jax.nn.softmax(
            (jnp.asarray(c_logits) * precision_q_cls + _gumbel(kg1, (B, K))) / TEMP,
            axis=-1,
        )
        g2 = _gumbel(kg2, (B, N, K, S))

        booksj = jnp.asarray(books)
        zej = jnp.asarray(ze)
        pq = precision_q

        b2 = jnp.sum(booksj * booksj, axis=-1)  # (K, S)
        # biasc[b, k, p, nch, s] = 2*g2[b, nch*128+p, k, s] - 2pq*b2[k, s]
        biasc = (2.0 * g2 - 2.0 * pq * b2[None, None]).reshape(B, NCH, 128, K, S)
        biasc = jnp.transpose(biasc, (0, 3, 2, 1, 4))  # (B, K, 128, NCH, S)

        zeT = (4.0 * pq) * jnp.transpose(zej, (0, 2, 1))  # (B, D, N)
        zeT = zeT.reshape(B, D // 128, 128, N)

        booksT = jnp.transpose(booksj, (2, 0, 1)).reshape(D // 128, 128, K * S)

        wbooks = jnp.einsum("bk,ksd->bds", c_probs, booksj)  # (B, D, S)
        wbw = (0.5 * wbooks).reshape(B, D // 128, 128, S)
        wbrow = -pq * (c_probs @ b2)  # (B, S)
        cvec = jnp.broadcast_to(c_probs[:, None, :], (B, 128, K))

        outs = jax.device_get(
            (biasc, zeT, booksT, wbw, wbrow, cvec, precision_q)
        )
    biasc, zeT, booksT, wbw, wbrow, cvec, precision_q = outs
    booksN = np.ascontiguousarray(
        books.reshape(K, S // 128, 128, D)
    ).astype(ml_dtypes.bfloat16)
    return dict(
        biasc=np.ascontiguousarray(biasc),
        zeT=np.ascontiguousarray(zeT),
        booksT=np.ascontiguousarray(booksT),
        booksN=booksN,
        wbw=np.ascontiguousarray(wbw),
        wbrow=np.ascontiguousarray(wbrow),
        cvec=np.ascontiguousarray(cvec),
        precision_q=np.float32(precision_q),
    )


def _enable_trace():
    """Wire up the NTFF profiling hook that the agent image leaves unplugged."""
    import types

    import antenv

    if "antenv.axon_hooks" not in sys.modules:
        from trn_agent_boot.trn_boot import _ntff_profile_via_ctypes

        hook = _ntff_profile_via_ctypes("/opt/axon/libaxon_pjrt.so")
        mod = types.ModuleType("antenv.axon_hooks")
        mod.get_axon_ntff_profile_hook = lambda: hook
        mod.set_axon_ntff_profile_hook = lambda h: None
        sys.modules["antenv.axon_hooks"] = mod
        antenv.axon_hooks = mod
    import concourse.bass_utils as bu

    bu.upload_artifacts = lambda tmpdir: tmpdir


# ---------------------------------------------------------------- entry point
def kernel(ze, c_logits, books, log_param_q, log_param_q_cls, is_train,
           _trace=False):
    from concourse.bass_utils import run_bass_kernel_spmd

    if _trace:
        _enable_trace()

    ze = np.asarray(ze, np.float32)
    c_logits = np.asarray(c_logits, np.float32)
    books = np.asarray(books, np.float32)

    prep = host_prep(ze, c_logits, books, log_param_q, log_param_q_cls)

    if "nc" not in _CACHE:
        _CACHE["nc"] = build_bass()
    nc = _CACHE["nc"]

    in_maps = []
    for c in range(NCORES):
        bs = slice(c * BL, (c + 1) * BL)
        in_maps.append(
            {
                "zeT": prep["zeT"][bs],
                "booksT": prep["booksT"],
                "booksN": prep["booksN"],
                "biasc": prep["biasc"][bs],
                "wbw": prep["wbw"][bs],
                "wbrow": prep["wbrow"][bs],
                "cvec": prep["cvec"][bs],
                "ones": np.ones((1, 128), np.float32),
            }
        )

    res = run_bass_kernel_spmd(nc, in_maps, list(range(NCORES)), trace=_trace)
    zq = np.concatenate([res.results[c]["zq"] for c in range(NCORES)], axis=0)
    prob = np.concatenate([res.results[c]["prob"] for c in range(NCORES)], axis=0)
    log_prob = np.concatenate(
        [res.results[c]["log_prob"] for c in range(NCORES)], axis=0
    )
    if _trace:
        _CACHE["last_exec_time_ns"] = res.exec_time_ns
        _CACHE["last_results"] = res
    return zq, prep["precision_q"], prob, log_prob
